# revision 1
# baseline (speedup 1.0000x reference)
"""Trainium2 Bass kernel for nn_GroupedQueryAttention_86380382257377.

Math note: the reference einsums collapse dramatically.
  scores = einsum('bqghd,bkgd->bqhg', q, k)  reduces over BOTH key pos and d,
  so only ksum[b,g,:] = sum_s k[b,s,g,:] is needed:
      scores[b,q,h,g] = x[b,q,:] . (Wq_blk[g,h] @ ksum[b,g]) / sqrt(D)
  out = einsum('bqhg,bsgd->bsgd', w, v) = wsum[b,g] * v[b,s,g,:]
  with wsum[b,g] = sum_{q,h} softmax_g(scores)[b,q,h,g], so
      out[b] = x[b] @ M[b] + cvec[b],
      M[b]   = sum_g wsum[b,g] * (Wv_g @ Wo_g),
      cvec[b]= sum_g wsum[b,g] * (bv_g @ Wo_g) + bo.

Sharding over 8 cores: core c owns group c for the Wq/Wk shards (one small
AllGather of the [D,B,H] wq_eff vectors) and owns output columns
[c*64,(c+1)*64) for the Wv@Wo / x@M stage (xT and WvT replicated).
"""

import numpy as np

B, S, D, G, H = 2, 2048, 512, 8, 4
N_CORES = 8
FSL = D // N_CORES  # 64 output columns per core
P = 128
DC = D // P  # 4
JC = S // P  # 16  (128-row score chunks over the full sequence)
SC = S // 512  # 4  (512-col moving chunks for the out matmul)
INV_SQRT_D = 1.0 / float(np.sqrt(D))

_cache = {}


def _build_nc():
    import concourse.bass as bass
    import concourse.mybir as mybir
    import concourse.tile as tile
    from concourse import bacc

    dt = mybir.dt.float32
    nc = bacc.Bacc(None, num_devices=N_CORES)

    # ---- kernel I/O (per-core views, host-prepared) ----
    xT_d = nc.dram_tensor("xT", [D, B, S], dt, kind="ExternalInput")      # [a, b, s]
    wvT_d = nc.dram_tensor("wvT", [G, D, D], dt, kind="ExternalInput")    # [g, e, a]
    wo_d = nc.dram_tensor("wo_sl", [G, D, FSL], dt, kind="ExternalInput")  # [g, e, f]
    wq_d = nc.dram_tensor("wqT", [D, H, D], dt, kind="ExternalInput")     # [e, h, a]
    wk_d = nc.dram_tensor("wk", [D, D], dt, kind="ExternalInput")         # [d, e]
    bk_d = nc.dram_tensor("bk_c", [D], dt, kind="ExternalInput")
    bq_d = nc.dram_tensor("bq_c", [H * D], dt, kind="ExternalInput")      # [h*512+e]
    bv_d = nc.dram_tensor("bv", [G * D], dt, kind="ExternalInput")
    bo_d = nc.dram_tensor("bo_sl", [FSL], dt, kind="ExternalInput")
    out_d = nc.dram_tensor("outT", [B, FSL, S], dt, kind="ExternalOutput")

    with tile.TileContext(nc) as tc:
        with (
            tc.tile_pool(name="sing", bufs=1) as sing,
            tc.tile_pool(name="wvp", bufs=2) as wvp,
            tc.tile_pool(name="pp", bufs=3, space="PSUM") as pp,
            tc.tile_pool(name="ppP", bufs=4, space="PSUM") as ppP,
            tc.tile_pool(name="dram", bufs=1, space="DRAM") as dram,
        ):
            # ---- persistent SBUF tiles ----
            x_sb = sing.tile([P, DC, B, S], dt)          # 8MB  [a_p, ac, b, s]
            wq_sb = sing.tile([P, DC, H, D], dt)         # 4MB  [e_p, ec, h, a]
            wo_sb = sing.tile([P, G, DC, FSL], dt)       # 1MB  [e_p, g, ec, f]
            wk_sb = sing.tile([P, DC, D], dt)            # 1MB  [d_p, dc, e]
            p_sb = sing.tile([P, DC, G, FSL], dt)        # 1MB  [a_p, ac, g, f]
            m_sb = sing.tile([P, DC, B, FSL], dt)        # .5MB [a_p, ac, b, f]
            out_sb = sing.tile([FSL, B, S], dt)          # 1MB  [f, b, s]
            wqe_all = sing.tile([P, DC, B, G, H], dt)    # .5MB [a_p, ac, b, g, h]
            s1_sb = sing.tile([P, B, JC, G, H], dt)      # .5MB scratch
            s2_sb = sing.tile([P, B, JC, G, H], dt)      # .5MB weights
            tmax = sing.tile([P, B, JC, H], dt)
            tden = sing.tile([P, B, JC, H], dt)
            trec = sing.tile([P, B, JC, H], dt)
            xs_sb = sing.tile([P, DC, B], dt)
            ksum_sb = sing.tile([P, DC, B], dt)          # [e_p, ec, b]
            bk_sb = sing.tile([P, DC], dt)
            bq_sb = sing.tile([P, DC, H], dt)            # [e_p, ec, h]
            bv_sb = sing.tile([P, G * DC], dt)           # [ge_p, ec32]
            bvs_sb = sing.tile([P, B, G * DC], dt)
            bo_sb = sing.tile([FSL, 1], dt)
            ones_sb = sing.tile([P, 1], dt)
            wsum_sb = sing.tile([1, B, G], dt)
            wsum_bc = sing.tile([P, B, G], dt)
            bqd_bc = sing.tile([P, B, G, H], dt)
            cvec_sb = sing.tile([FSL, B], dt)

            # ---- internal DRAM (collective bounce + broadcast) ----
            CHUNK = D * B * H + H * B  # 4096 wq_eff + 8 bq_dot
            wq_bounce = dram.tile([CHUNK], dt)
            wq_gath = dram.tile([G * CHUNK], dt)
            wsum_dd = dram.tile([B, G], dt)

            # ---- input DMAs (priority order = program order) ----
            for dc in range(DC):
                nc.sync.dma_start(
                    out=x_sb[:, dc, :, :], in_=xT_d[dc * P:(dc + 1) * P, :, :]
                )
            nc.sync.dma_start(
                out=wk_sb[:, :, :],
                in_=wk_d.rearrange("(dc p) e -> p dc e", p=P),
            )
            nc.sync.dma_start(
                out=bk_sb[:, :], in_=bk_d.rearrange("(ec p) -> p ec", p=P)
            )
            for h in range(H):
                nc.sync.dma_start(
                    out=bq_sb[:, :, h],
                    in_=bq_d[h * D:(h + 1) * D].rearrange("(ec p) -> p ec", p=P),
                )
            nc.sync.dma_start(
                out=wq_sb[:, :, :, :],
                in_=wq_d.rearrange("(ec p) h a -> p ec h a", p=P),
            )
            for g in range(G):
                nc.sync.dma_start(
                    out=wo_sb[:, g, :, :],
                    in_=wo_d[g, :, :].rearrange("(ec p) f -> p ec f", p=P),
                )
            nc.sync.dma_start(
                out=bv_sb[:, :], in_=bv_d.rearrange("(ec p) -> p ec", p=P)
            )
            nc.sync.dma_start(
                out=bo_sb[:, :], in_=bo_d.rearrange("(f o) -> f o", o=1)
            )
            nc.vector.memset(ones_sb[:, :], 1.0)

            # ---- A. xs[b,d] = sum_s x  (reduce innermost S) ----
            for dc in range(DC):
                nc.vector.tensor_reduce(
                    out=xs_sb[:, dc, :],
                    in_=x_sb[:, dc, :, :],
                    axis=mybir.AxisListType.X,
                    op=mybir.AluOpType.add,
                )

            # ---- B. ksumT[e,b] = Wk_c^T xs + S*bk  ----
            nc.vector.tensor_scalar_mul(bk_sb[:, :], bk_sb[:, :], float(S))
            psum_k = pp.tile([P, DC, B], dt, tag="big")
            for ec in range(DC):
                for dc in range(DC):
                    nc.tensor.matmul(
                        psum_k[:, ec, :],
                        lhsT=wk_sb[:, dc, ec * P:(ec + 1) * P],
                        rhs=xs_sb[:, dc, :],
                        start=(dc == 0),
                        stop=(dc == DC - 1),
                    )
            for ec in range(DC):
                nc.vector.tensor_scalar_add(
                    ksum_sb[:, ec, :], psum_k[:, ec, :], bk_sb[:, ec:ec + 1]
                )

            # ---- C. wq_eff[a,(b)] per (h, ac); bq_dot[h,b] ----
            psum_wq = pp.tile([P, H, DC, B], dt, tag="big")
            for h in range(H):
                for ac in range(DC):
                    for ec in range(DC):
                        nc.tensor.matmul(
                            psum_wq[:, h, ac, :],
                            lhsT=wq_sb[:, ec, h, ac * P:(ac + 1) * P],
                            rhs=ksum_sb[:, ec, :],
                            start=(ec == 0),
                            stop=(ec == DC - 1),
                        )
            psum_bqd = pp.tile([B, H], dt, tag="big")
            for ec in range(DC):
                nc.tensor.matmul(
                    psum_bqd[:, :],
                    lhsT=ksum_sb[:, ec, :],
                    rhs=bq_sb[:, ec, :],
                    start=(ec == 0),
                    stop=(ec == DC - 1),
                )
            # stage psum -> sbuf (layout [p, ac, b, h]) -> flat dram bounce
            wqe_loc = sing.tile([P, DC, B, H], dt)
            bqd_loc = sing.tile([B, H], dt)
            nc.vector.tensor_copy(
                wqe_loc[:, :, :, :].rearrange("p ac b h -> p h ac b"),
                psum_wq[:, :, :, :],
            )
            nc.vector.tensor_copy(bqd_loc[:, :], psum_bqd[:, :])
            nc.sync.dma_start(
                out=wq_bounce[0:D * B * H].rearrange(
                    "(p ac b h) -> p ac b h", p=P, ac=DC, b=B
                ),
                in_=wqe_loc[:, :, :, :],
            )
            nc.sync.dma_start(
                out=wq_bounce[D * B * H:CHUNK].rearrange("(b h) -> b h", b=B),
                in_=bqd_loc[:, :],
            )

            # ---- D. AllGather of (wq_eff, bq_dot) ----
            nc.gpsimd.collective_compute(
                "AllGather",
                mybir.AluOpType.bypass,
                replica_groups=[list(range(N_CORES))],
                ins=[wq_bounce[:].opt()],
                outs=[wq_gath[:].opt()],
            )

            # ---- E. spread gathered results ----
            gap = wq_gath[:]
            for b in range(B):
                for ac in range(DC):
                    nc.sync.dma_start(
                        out=wqe_all[:, ac, b, :, :].opt(),
                        in_=bass.AP(
                            tensor=gap.tensor,
                            offset=gap.offset + ac * B * H + b * H,
                            ap=[[DC * B * H, P], [CHUNK, G], [1, H]],
                        ),
                    )
            for b in range(B):
                nc.sync.dma_start(
                    out=bqd_bc[:, b, :, :],
                    in_=bass.AP(
                        tensor=gap.tensor,
                        offset=gap.offset + D * B * H + b * H,
                        ap=[[0, P], [CHUNK, G], [1, H]],
                    ),
                )
            nc.vector.tensor_scalar_mul(
                bqd_bc[:, :, :, :], bqd_bc[:, :, :, :], INV_SQRT_D
            )

            # ---- F. scores + softmax + wsum (full sequence, every core) ----
            for b in range(B):
                psum_s = pp.tile([P, JC, G, H], dt, tag="big")
                for j in range(JC):
                    for dc in range(DC):
                        nc.tensor.matmul(
                            psum_s[:, j, :, :],
                            lhsT=x_sb[:, dc, b, j * P:(j + 1) * P],
                            rhs=wqe_all[:, dc, b, :, :],
                            start=(dc == 0),
                            stop=(dc == DC - 1),
                        )
                # t = scores*inv_sqrt_d + bqd   (into s1)
                bqd_b = bqd_bc[:, b, :, :]
                nc.vector.scalar_tensor_tensor(
                    out=s1_sb[:, b, :, :, :],
                    in0=psum_s[:, :, :, :],
                    scalar=INV_SQRT_D,
                    in1=bass.AP(
                        tensor=bqd_b.tensor,
                        offset=bqd_b.offset,
                        ap=[list(bqd_b.ap[0]), [0, JC]] + list(bqd_b.ap[1:]),
                    ),
                    op0=mybir.AluOpType.mult,
                    op1=mybir.AluOpType.add,
                )
                # row max over g (innermost via stride permute)
                nc.vector.tensor_reduce(
                    out=tmax[:, b, :, :],
                    in_=s1_sb[:, b, :, :, :].rearrange("p j g h -> p j h g"),
                    axis=mybir.AxisListType.X,
                    op=mybir.AluOpType.max,
                )
                tmax_b = tmax[:, b, :, :]
                nc.vector.tensor_tensor(
                    out=s2_sb[:, b, :, :, :].rearrange("p j g h -> p j h g"),
                    in0=s1_sb[:, b, :, :, :].rearrange("p j g h -> p j h g"),
                    in1=bass.AP(
                        tensor=tmax_b.tensor,
                        offset=tmax_b.offset,
                        ap=list(tmax_b.ap) + [[0, G]],
                    ),
                    op=mybir.AluOpType.subtract,
                )
                nc.scalar.activation(
                    out=s1_sb[:, b, :, :, :],
                    in_=s2_sb[:, b, :, :, :],
                    func=mybir.ActivationFunctionType.Exp,
                )
                nc.vector.tensor_reduce(
                    out=tden[:, b, :, :],
                    in_=s1_sb[:, b, :, :, :].rearrange("p j g h -> p j h g"),
                    axis=mybir.AxisListType.X,
                    op=mybir.AluOpType.add,
                )
                nc.vector.reciprocal(trec[:, b, :, :], tden[:, b, :, :])
                trec_b = trec[:, b, :, :]
                nc.vector.tensor_tensor(
                    out=s2_sb[:, b, :, :, :].rearrange("p j g h -> p j h g"),
                    in0=s1_sb[:, b, :, :, :].rearrange("p j g h -> p j h g"),
                    in1=bass.AP(
                        tensor=trec_b.tensor,
                        offset=trec_b.offset,
                        ap=list(trec_b.ap) + [[0, G]],
                    ),
                    op=mybir.AluOpType.mult,
                )
                # wsum partial: ones^T @ weights -> [1, JC*G*H], reduce (j,h)
                psum_ws = pp.tile([1, JC * G * H], dt, tag="big")
                nc.tensor.matmul(
                    psum_ws[:, :],
                    lhsT=ones_sb[:, :],
                    rhs=s2_sb[:, b, :, :, :],
                    start=True,
                    stop=True,
                )
                # view [1, (g), (j), (h)] with g kept, (j,h) reduced
                psv = psum_ws[:, :].rearrange(
                    "p (j g h) -> p g j h", j=JC, g=G, h=H
                )
                nc.vector.tensor_reduce(
                    out=wsum_sb[:, b, :],
                    in_=psv,
                    axis=mybir.AxisListType.XY,
                    op=mybir.AluOpType.add,
                )

            # broadcast wsum to all partitions via DRAM
            nc.sync.dma_start(out=wsum_dd[:, :], in_=wsum_sb[:, :, :])
            wsrc = wsum_dd[:, :]
            nc.sync.dma_start(
                out=wsum_bc[:, :, :],
                in_=bass.AP(
                    tensor=wsrc.tensor,
                    offset=wsrc.offset,
                    ap=[[0, P]] + list(wsrc.ap),
                ),
            )

            # ---- G. P_g = Wv_g @ Wo_g[:, fsl]  (all groups, f-slice) ----
            for g in range(G):
                wv_g = wvp.tile([P, DC, D], dt)
                nc.sync.dma_start(
                    out=wv_g[:, :, :],
                    in_=wvT_d[g, :, :].rearrange("(ec p) a -> p ec a", p=P),
                )
                for ac in range(DC):
                    psum_p = ppP.tile([P, FSL], dt)
                    for ec in range(DC):
                        nc.tensor.matmul(
                            psum_p[:, :],
                            lhsT=wv_g[:, ec, ac * P:(ac + 1) * P],
                            rhs=wo_sb[:, g, ec, :],
                            start=(ec == 0),
                            stop=(ec == DC - 1),
                        )
                    nc.vector.tensor_copy(p_sb[:, ac, g, :], psum_p[:, :])

            # ---- H. M[b] = sum_g wsum[b,g] * P_g ----
            for b in range(B):
                nc.vector.tensor_scalar_mul(
                    m_sb[:, :, b, :], p_sb[:, :, 0, :], wsum_bc[:, b, 0:1]
                )
                for g in range(1, G):
                    nc.vector.scalar_tensor_tensor(
                        out=m_sb[:, :, b, :],
                        in0=p_sb[:, :, g, :],
                        scalar=wsum_bc[:, b, g:g + 1],
                        in1=m_sb[:, :, b, :],
                        op0=mybir.AluOpType.mult,
                        op1=mybir.AluOpType.add,
                    )

            # ---- I. cvec[b] = sum_g wsum[b,g] * (bv_g @ Wo_g[:,fsl]) + bo ----
            for b in range(B):
                wsb = wsum_bc[:, b, :]
                nc.vector.tensor_tensor(
                    out=bvs_sb[:, b, :].rearrange("p (g r) -> p g r", g=G),
                    in0=bv_sb[:, :].rearrange("p (g r) -> p g r", g=G),
                    in1=bass.AP(
                        tensor=wsb.tensor,
                        offset=wsb.offset,
                        ap=list(wsb.ap) + [[0, DC]],
                    ),
                    op=mybir.AluOpType.mult,
                )
                psum_cv = pp.tile([FSL, 1], dt, tag="big")
                for ec32 in range(G * DC):
                    nc.tensor.matmul(
                        psum_cv[:, :],
                        lhsT=wo_sb[:, ec32 // DC, ec32 % DC, :],
                        rhs=bvs_sb[:, b, ec32:ec32 + 1],
                        start=(ec32 == 0),
                        stop=(ec32 == G * DC - 1),
                    )
                nc.vector.tensor_tensor(
                    out=cvec_sb[:, b:b + 1],
                    in0=psum_cv[:, :],
                    in1=bo_sb[:, :],
                    op=mybir.AluOpType.add,
                )

            # ---- J. outT[b] = (x[b] @ M[b])^T + cvec ----
            for b in range(B):
                for sc in range(SC):
                    psum_o = pp.tile([FSL, 512], dt, tag="big")
                    for ac in range(DC):
                        nc.tensor.matmul(
                            psum_o[:, :],
                            lhsT=m_sb[:, ac, b, :],
                            rhs=x_sb[:, ac, b, sc * 512:(sc + 1) * 512],
                            start=(ac == 0),
                            stop=(ac == DC - 1),
                        )
                    nc.vector.tensor_scalar_add(
                        out_sb[:, b, sc * 512:(sc + 1) * 512],
                        psum_o[:, :],
                        cvec_sb[:, b:b + 1],
                    )
                nc.sync.dma_start(out=out_d[b, :, :], in_=out_sb[:, b, :])

    nc.compile()
    return nc


def kernel(x, Wq, bq, Wk, bk, Wv, bv, Wo, bo):
    from concourse.bass_utils import run_bass_kernel_spmd

    if "nc" not in _cache:
        _cache["nc"] = _build_nc()
    nc = _cache["nc"]

    x = np.ascontiguousarray(x, dtype=np.float32)
    xT = np.ascontiguousarray(x.transpose(2, 0, 1))                    # [D,B,S]
    wvT = np.ascontiguousarray(
        Wv.astype(np.float32).reshape(D, G, D).transpose(1, 2, 0)      # [g,e,a]
    )
    wo_r = Wo.astype(np.float32).reshape(G, D, D)
    wq_r = Wq.astype(np.float32).reshape(D, G, H, D)
    bq_r = np.ascontiguousarray(bq, dtype=np.float32)
    in_maps = []
    for c in range(N_CORES):
        fs = slice(c * FSL, (c + 1) * FSL)
        in_maps.append({
            "xT": xT,
            "wvT": wvT,
            "wo_sl": np.ascontiguousarray(wo_r[:, :, fs]),
            "wqT": np.ascontiguousarray(wq_r[:, c].transpose(2, 1, 0)),  # [e,h,a]
            "wk": np.ascontiguousarray(Wk[:, c * D:(c + 1) * D].astype(np.float32)),
            "bk_c": np.ascontiguousarray(bk[c * D:(c + 1) * D].astype(np.float32)),
            "bq_c": np.ascontiguousarray(bq_r[c * H * D:(c + 1) * H * D]),
            "bv": np.ascontiguousarray(bv, dtype=np.float32),
            "bo_sl": np.ascontiguousarray(bo[fs].astype(np.float32)),
        })
    res = run_bass_kernel_spmd(nc, in_maps, core_ids=list(range(N_CORES)))
    _cache["last_results"] = res
    outs = [r["outT"] for r in res.results]          # each [B, FSL, S]
    full = np.concatenate(outs, axis=1)              # [B, D, S]
    return np.ascontiguousarray(full.transpose(0, 2, 1)).astype(np.float32)



# revision 14
# speedup vs baseline: 1.9862x; 1.9862x over previous
"""Trainium2 Bass kernel for nn_GroupedQueryAttention_86380382257377.

Math: the reference einsums collapse.
  scores[b,q,h,g] = x[b,q,:] . (Wq_blk[g,h] @ ksum[b,g]) / sqrt(D)
  with ksum[b,g,:] = sum_s k[b,s,g,:];  weights = softmax_g(scores)
  out[b] = x[b] @ M[b] + cvec[b],
      M[b]   = sum_g wsum[b,g] * (Wv_g @ Wo_g),
      wsum[b,g] = sum_{q,h} weights[b,q,h,g]
      cvec[b]= sum_g wsum[b,g] * (bv_g @ Wo_g) + bo  (host-applied, exact).

Sharding: core c owns group c for Wq/Wk (one small AllGather of the
wq_eff vectors) and output columns [c*64,(c+1)*64) for Wv@Wo / x@M
(x, Wv replicated).

Precision: the scores path runs fp8(e4m3) with host-side scaling
(Wq*256 dodges subnormals, ksum/8 and wqe/64 dodge overflow; the exp
scale absorbs the net factor). Stage J / P use bf16. fp32 PSUM
everywhere. Verified ~7e-3 rel err in numpy (gate 2e-2).
"""

import numpy as np
import ml_dtypes

B, S, D, G, H = 2, 2048, 512, 8, 4
N_CORES = 8
FSL = D // N_CORES  # 64
P = 128
DC = D // P  # 4
JC = S // P  # 16
ACBH = DC * B * H  # 32
WQE = P * ACBH  # 4096
CHUNK = WQE + B * H + 8  # 4112 elems, 32B-aligned in bf16
INV_SQRT_D = 1.0 / float(np.sqrt(D))
EXP_SCALE = 2.0 * INV_SQRT_D  # wqe8 carries wq_eff/2
WQ_UP = 256.0
KS_DN = 1.0 / 8.0
WQE_DN = 1.0 / 64.0

_cache = {}


def _build_nc():
    import concourse.bass as bass
    import concourse.mybir as mybir
    import concourse.tile as tile
    from concourse import bacc

    fp = mybir.dt.float32
    bf = mybir.dt.bfloat16
    f8 = mybir.dt.float8e4
    nc = bacc.Bacc(None, num_devices=N_CORES)

    # ---- kernel I/O (per-core views, host-prepared) ----
    x8_d = nc.dram_tensor("x8T", [D, B, S], f8, kind="ExternalInput")      # [a,b,s]
    xb_d = nc.dram_tensor("xbT", [D, B, S], bf, kind="ExternalInput")      # [a,b,s]
    wq_d = nc.dram_tensor("wq8T", [D, H, D], f8, kind="ExternalInput")     # [e,h,a]
    wk_d = nc.dram_tensor("wk", [D, D], bf, kind="ExternalInput")          # [d,e]
    wv_d = nc.dram_tensor("wvT", [G, D, D], bf, kind="ExternalInput")      # [g,e,a]
    wo_d = nc.dram_tensor("wo_img", [P, G * DC * FSL], bf, kind="ExternalInput")
    bk_d = nc.dram_tensor("bk_img", [P, DC], fp, kind="ExternalInput")     # *S
    bq_d = nc.dram_tensor("bq_img", [P, DC * H], fp, kind="ExternalInput")
    out_d = nc.dram_tensor("out2", [B, S, FSL], bf, kind="ExternalOutput")
    ws_d = nc.dram_tensor("wsum_out", [B, G], fp, kind="ExternalOutput")

    with tile.TileContext(nc) as tc:
        with (
            tc.tile_pool(name="sing", bufs=1) as sing,
            tc.tile_pool(name="pk", bufs=1, space="PSUM") as pk,
            tc.tile_pool(name="ps", bufs=2, space="PSUM") as ps,
            tc.tile_pool(name="pw", bufs=1, space="PSUM") as pw,
            tc.tile_pool(name="ppP", bufs=2, space="PSUM") as ppP,
            tc.tile_pool(name="po", bufs=2, space="PSUM") as po,
            tc.tile_pool(name="dram", bufs=1, space="DRAM") as dram,
        ):
            # ---- persistent SBUF tiles ----
            x8_sb = sing.tile([P, DC, B, S], f8)         # 16KB/part
            xb_sb = sing.tile([P, DC, B, S], bf)         # 32KB/part
            wq_sb = sing.tile([P, DC, H, D], f8)         # 8KB/part
            wk_sb = sing.tile([P, DC, D], bf)            # 4KB/part
            wv_sb = sing.tile([P, G, DC, D], bf)         # 32KB/part
            wo_sb = sing.tile([P, G, DC, FSL], bf)       # 4KB/part
            bk_sb = sing.tile([P, DC], fp)
            bq_sb = sing.tile([P, DC, H], fp)
            red8 = sing.tile([P, S], f8)                 # Act-accum scratch
            xs_f = sing.tile([P, DC, B], fp)
            xs_bf = sing.tile([P, DC, B], bf)
            ksum_f = sing.tile([P, DC, B], fp)
            ksum8 = sing.tile([P, DC, B], f8)
            wqe_loc = sing.tile([P, DC, B, H], bf)
            bqd_loc = sing.tile([B, H + 4], bf)  # cols 4-7 zero-pad CHUNK
            wqe_all = sing.tile([P, G, DC, B, H], bf)
            wqe8 = sing.tile([P, G, DC, B, H], f8)
            bqd1 = sing.tile([1, G, B, H], bf)
            ebqd1 = sing.tile([1, B, H, G], bf)
            ebqd_bc = sing.tile([P, B, H, G], bf)
            e_sb = sing.tile([P, B, JC, H, G], bf)
            e2_sb = sing.tile([P, B, JC, H, G], bf)
            den_sb = sing.tile([P, B, JC, H], bf)
            rec_sb = sing.tile([P, B, JC, H], bf)
            ws_sb = sing.tile([1, B, G], fp)
            wsum_bc = sing.tile([P, B, G], fp)
            p_sb = sing.tile([P, DC, G, FSL], bf)
            m_sb = sing.tile([P, DC, B, FSL], bf)
            out_sb = sing.tile([P, JC, B, FSL], bf)      # [s%128, j, b, f]
            ones_f = sing.tile([1, P], fp)
            ones_b = sing.tile([1, P], bf)

            bounce = dram.tile([CHUNK], bf)
            gath = dram.tile([G * CHUNK], bf)

            nc.vector.memset(ones_f[:, :], 1.0)
            nc.vector.memset(bqd_loc[:, :], 0.0)
            nc.vector.memset(ones_b[:, :], 1.0)

            # ---- input DMAs on SP queue (program order = priority) ----
            nc.sync.dma_start(
                out=wk_sb[:, :, :], in_=wk_d.rearrange("(dc p) e -> p dc e", p=P)
            )
            nc.sync.dma_start(out=bk_sb[:, :], in_=bk_d[:, :])
            nc.sync.dma_start(
                out=bq_sb[:, :, :],
                in_=bq_d.rearrange("p (ec h) -> p ec h", ec=DC),
            )
            for dc in range(DC):
                for b in range(B):
                    nc.sync.dma_start(
                        out=x8_sb[:, dc, b, :],
                        in_=x8_d[dc * P:(dc + 1) * P, b, :],
                    )
            for ec in range(DC):
                nc.sync.dma_start(
                    out=wq_sb[:, ec, :, :], in_=wq_d[ec * P:(ec + 1) * P, :, :]
                )
            nc.sync.dma_start(
                out=wo_sb[:, :, :, :],
                in_=wo_d.rearrange("p (g ec f) -> p g ec f", g=G, ec=DC),
            )
            for g in range(G):
                for eh in range(2):
                    nc.sync.dma_start(
                        out=wv_sb[:, g, 2 * eh:2 * eh + 2, :],
                        in_=wv_d[g, 256 * eh:256 * (eh + 1), :].rearrange(
                            "(ec p) a -> p ec a", p=P
                        ),
                    )
            for b in range(B):
                for dc in range(DC):
                    for sh in range(2):
                        nc.sync.dma_start(
                            out=xb_sb[:, dc, b, 1024 * sh:1024 * (sh + 1)],
                            in_=xb_d[
                                dc * P:(dc + 1) * P, b, 1024 * sh:1024 * (sh + 1)
                            ],
                        )

            # ---- A. xs[a,b] = sum_s x8 across DVE (fused ttr) / Act lanes ----
            ttr_scr = sing.tile([P, S // 2], bf)  # ttr dummy out
            def reduce_unit(eng, dc, b, slot):
                nc.vector.tensor_reduce(
                    out=slot, in_=x8_sb[:, dc, b, :],
                    axis=mybir.AxisListType.X, op=mybir.AluOpType.add,
                )

            for dc in range(DC):
                for b in range(B):
                    reduce_unit("dve", dc, b, xs_f[:, dc, b:b + 1])
            with nc.allow_low_precision(reason="bf16 xs for bf16 matmul"):
                nc.vector.tensor_copy(xs_bf[:, :, :], xs_f[:, :, :])

            # ---- B. ksumT[e,b] = Wk_c^T xs + S*bk ----
            psum_k = pk.tile([P, DC, B], fp, tag="pk")
            for ec in range(DC):
                for dc in range(DC):
                    nc.tensor.matmul(
                        psum_k[:, ec, :],
                        lhsT=wk_sb[:, dc, ec * P:(ec + 1) * P],
                        rhs=xs_bf[:, dc, :],
                        start=(dc == 0),
                        stop=(dc == DC - 1),
                    )
            for ec in range(DC):
                nc.vector.tensor_scalar_add(
                    ksum_f[:, ec, :], psum_k[:, ec, :], bk_sb[:, ec:ec + 1]
                )
            with nc.allow_low_precision(reason="fp8 ksum for fp8 matmul, /8"):
                nc.vector.tensor_scalar_mul(ksum8[:, :, :], ksum_f[:, :, :], KS_DN)

            # ---- C. wq_eff (x32 scaled) per (h, ac); bq_dot[b,h] exact ----
            psum_wq = pk.tile([P, H, DC, B], fp, tag="pk")
            for h in range(H):
                for ac in range(DC):
                    for ec in range(DC):
                        nc.tensor.matmul(
                            psum_wq[:, h, ac, :],
                            lhsT=wq_sb[:, ec, h, ac * P:(ac + 1) * P],
                            rhs=ksum8[:, ec, :],
                            start=(ec == 0),
                            stop=(ec == DC - 1),
                        )
            psum_bqd = pk.tile([B, H], fp, tag="pk")
            for ec in range(DC):
                nc.tensor.matmul(
                    psum_bqd[:, :],
                    lhsT=ksum_f[:, ec, :],
                    rhs=bq_sb[:, ec, :],
                    start=(ec == 0),
                    stop=(ec == DC - 1),
                )
            nc.vector.tensor_copy(
                wqe_loc[:, :, :, :].rearrange("p ac b h -> p h ac b"),
                psum_wq[:, :, :, :],
            )
            nc.vector.tensor_copy(bqd_loc[:, 0:H], psum_bqd[:, :])
            # bounce out + AllGather + spread (all on Pool queue, in order)
            nc.scalar.dma_start(
                out=bounce[0:WQE].rearrange("(p f) -> p f", p=P),
                in_=wqe_loc[:, :, :, :],
            )
            nc.scalar.dma_start(
                out=bounce[WQE:WQE + B * (H + 4)].rearrange(
                    "(b h) -> b h", b=B
                ),
                in_=bqd_loc[:, :],
            )
            nc.gpsimd.collective_compute(
                "AllGather",
                mybir.AluOpType.bypass,
                replica_groups=[list(range(N_CORES))],
                ins=[bounce[:].opt()],
                outs=[gath[:].opt()],
            )
            gap = gath[:]
            nc.scalar.dma_start(
                out=wqe_all[:, :, :, :, :].opt(),
                in_=bass.AP(
                    tensor=gap.tensor,
                    offset=gap.offset,
                    ap=[[ACBH, P], [CHUNK, G], [1, ACBH]],
                ),
            )
            nc.scalar.dma_start(
                out=bqd1[:, :, :, :].opt(),
                in_=bass.AP(
                    tensor=gap.tensor,
                    offset=gap.offset + WQE,
                    ap=[[0, 1], [CHUNK, G], [1, B * H]],
                ),
            )
            with nc.allow_low_precision(reason="fp8 wqe for fp8 matmul, /64"):
                nc.vector.tensor_scalar_mul(
                    wqe8[:, :, :, :, :], wqe_all[:, :, :, :, :], WQE_DN
                )
            # ebqd = exp(bqd/sqrt(D)); broadcast to partitions via PE
            nc.scalar.activation(
                out=ebqd1[0:1, :, :, :],
                in_=bqd1[0:1, :, :, :].rearrange("p g b h -> p b h g"),
                func=mybir.ActivationFunctionType.Exp,
                scale=INV_SQRT_D,
            )
            psum_eb = pk.tile([P, B * H * G], fp, tag="pk")
            nc.tensor.matmul(
                psum_eb[:, :],
                lhsT=ones_b[0:1, :],
                rhs=ebqd1[0:1, :, :, :],
                start=True,
                stop=True,
            )
            nc.vector.tensor_copy(
                ebqd_bc[:, :, :, :],
                psum_eb[:, :].rearrange("p (b h g) -> p b h g", b=B, h=H),
            )

            # ---- G. P_g = Wv_g @ Wo_g[:, fsl] (fills the AllGather window) ----
            for g in range(G):
                psum_pg = ppP.tile([P, DC * FSL], fp, name="psum_pg")
                for ac in range(DC):
                    for ec in range(DC):
                        nc.tensor.matmul(
                            psum_pg[:, ac * FSL:(ac + 1) * FSL],
                            lhsT=wv_sb[:, g, ec, ac * P:(ac + 1) * P],
                            rhs=wo_sb[:, g, ec, :],
                            start=(ec == 0),
                            stop=(ec == DC - 1),
                        )
                pv = psum_pg[:, :].rearrange("p (ac f) -> p ac f", ac=DC)
                if g % 2 == 0:
                    nc.scalar.activation(
                        out=p_sb[:, :, g, :],
                        in_=pv,
                        func=mybir.ActivationFunctionType.Copy,
                    )
                else:
                    nc.vector.tensor_copy(p_sb[:, :, g, :], pv)

            # ---- F. scores + softmax + wsum, then M and out, per b ----
            psum_s = [
                ps.tile([P, JC, H, G], fp, tag="s", name=f"psum_s{b}")
                for b in range(B)
            ]
            psum_ws = [
                pw.tile([1, G], fp, tag="w", name=f"psum_ws{b}")
                for b in range(B)
            ]
            psum_wsb = [
                pk.tile([P, G], fp, tag="pk", name=f"psum_wsb{b}")
                for b in range(B)
            ]

            for b in range(B):
                for j in range(JC):
                    for dc in range(DC):
                        nc.tensor.matmul(
                            psum_s[b][:, j, :, :],
                            lhsT=x8_sb[:, dc, b, j * P:(j + 1) * P],
                            rhs=wqe8[:, :, dc, b, :].rearrange("p g h -> p h g"),
                            start=(dc == 0),
                            stop=(dc == DC - 1),
                        )
            for b in range(B):
                nc.scalar.activation(
                    out=e_sb[:, b, :, :, :],
                    in_=psum_s[b][:, :, :, :],
                    func=mybir.ActivationFunctionType.Exp,
                    scale=EXP_SCALE,
                )
                eb = ebqd_bc[:, b, :, :]
                nc.vector.tensor_tensor(
                    out=e2_sb[:, b, :, :, :],
                    in0=e_sb[:, b, :, :, :],
                    in1=bass.AP(
                        tensor=eb.tensor,
                        offset=eb.offset,
                        ap=[list(eb.ap[0]), [0, JC]] + list(eb.ap[1:]),
                    ),
                    op=mybir.AluOpType.mult,
                )
                with nc.allow_low_precision(reason="bf16 softmax den, ~7e-3 ok"):
                    nc.vector.tensor_reduce(
                        out=den_sb[:, b, :, :],
                        in_=e2_sb[:, b, :, :, :],
                        axis=mybir.AxisListType.X,
                        op=mybir.AluOpType.add,
                    )
                    nc.vector.reciprocal(rec_sb[:, b, :, :], den_sb[:, b, :, :])

            for b in range(B):
                # wsum[g] = sum_{p,j,h} rec[p,j,h] * e2[p,j,h,g] via PE accum
                n = 0
                for j in range(JC):
                    for h in range(H):
                        nc.tensor.matmul(
                            psum_ws[b][:, :],
                            lhsT=rec_sb[:, b, j, h:h + 1],
                            rhs=e2_sb[:, b, j, h, :],
                            start=(n == 0),
                            stop=(n == JC * H - 1),
                        )
                        n += 1
                nc.vector.tensor_copy(ws_sb[:, b, :], psum_ws[b][:, :])
                nc.tensor.matmul(
                    psum_wsb[b][:, :],
                    lhsT=ones_f[0:1, :],
                    rhs=ws_sb[0:1, b, :],
                    start=True,
                    stop=True,
                )
                nc.vector.tensor_copy(wsum_bc[:, b, :], psum_wsb[b][:, :])

                # ---- H. M[b] = sum_g wsum[b,g] * P_g ----
                nc.vector.tensor_scalar_mul(
                    m_sb[:, :, b, :], p_sb[:, :, 0, :], wsum_bc[:, b, 0:1]
                )
                for g in range(1, G):
                    nc.vector.scalar_tensor_tensor(
                        out=m_sb[:, :, b, :],
                        in0=p_sb[:, :, g, :],
                        scalar=wsum_bc[:, b, g:g + 1],
                        in1=m_sb[:, :, b, :],
                        op0=mybir.AluOpType.mult,
                        op1=mybir.AluOpType.add,
                    )

                # ---- J. out[b, s, f] = x[b] @ M[b], s-chunked [128, 64] ----
                for j in range(JC):
                    psum_o = po.tile([P, FSL], fp, name="psum_o")
                    for dc in range(DC):
                        nc.tensor.matmul(
                            psum_o[:, :],
                            lhsT=xb_sb[:, dc, b, j * P:(j + 1) * P],
                            rhs=m_sb[:, dc, b, :],
                            start=(dc == 0),
                            stop=(dc == DC - 1),
                        )
                    if j % 2 == 0:
                        nc.scalar.activation(
                            out=out_sb[:, j, b, :],
                            in_=psum_o[:, :],
                            func=mybir.ActivationFunctionType.Copy,
                        )
                    else:
                        nc.vector.tensor_copy(out_sb[:, j, b, :], psum_o[:, :])
                nc.sync.dma_start(
                    out=out_d[b, :, :].rearrange("(j p) f -> p j f", p=P),
                    in_=out_sb[:, :, b, :],
                )
            nc.sync.dma_start(out=ws_d[:, :], in_=ws_sb[0, :, :])

    nc.compile()
    return nc


def kernel(x, Wq, bq, Wk, bk, Wv, bv, Wo, bo):
    from concourse.bass_utils import run_bass_kernel_spmd

    if "nc" not in _cache:
        _cache["nc"] = _build_nc()
    nc = _cache["nc"]

    bft = ml_dtypes.bfloat16
    f8t = ml_dtypes.float8_e4m3fn
    x = np.ascontiguousarray(x, dtype=np.float32)
    xT = np.ascontiguousarray(x.transpose(2, 0, 1))                    # [D,B,S]
    x8T = xT.astype(f8t)
    xbT = xT.astype(bft)
    wvT = np.ascontiguousarray(
        Wv.astype(np.float32).reshape(D, G, D).transpose(1, 2, 0)      # [g,e,a]
    ).astype(bft)
    wo_r = Wo.astype(np.float32).reshape(G, D, D)
    wq_r = Wq.astype(np.float32).reshape(D, G, H, D)
    bq_r = np.asarray(bq, dtype=np.float32).reshape(G, H, D)
    bk_r = np.asarray(bk, dtype=np.float32).reshape(G, D)
    in_maps = []
    for c in range(N_CORES):
        fs = slice(c * FSL, (c + 1) * FSL)
        wo_img = np.ascontiguousarray(
            wo_r[:, :, fs].reshape(G, DC, P, FSL).transpose(2, 0, 1, 3)
        ).reshape(P, G * DC * FSL).astype(bft)
        bq_img = np.ascontiguousarray(
            bq_r[c].transpose(1, 0).reshape(DC, P, H).transpose(1, 0, 2)
        ).reshape(P, DC * H)  # [p, ec, h], e = ec*128+p
        bk_img = np.ascontiguousarray(bk_r[c].reshape(DC, P).T) * float(S)
        in_maps.append({
            "x8T": x8T,
            "xbT": xbT,
            "wq8T": np.ascontiguousarray(
                (WQ_UP * wq_r[:, c]).transpose(2, 1, 0)
            ).astype(f8t),
            "wk": np.ascontiguousarray(
                Wk[:, c * D:(c + 1) * D].astype(np.float32)
            ).astype(bft),
            "wvT": wvT,
            "wo_img": wo_img,
            "bk_img": np.ascontiguousarray(bk_img),
            "bq_img": bq_img,
        })
    res = run_bass_kernel_spmd(nc, in_maps, core_ids=list(range(N_CORES)))
    _cache["last_results"] = res
    outs = [r["out2"].astype(np.float32) for r in res.results]  # [B,S,FSL] each
    out = np.ascontiguousarray(np.concatenate(outs, axis=2))    # [B, S, D]
    # bias correction (exact; zero when bv == bo == 0)
    wsum = res.results[0]["wsum_out"].astype(np.float32)        # [B, G]
    pb = np.einsum(
        "gd,gdf->gf",
        np.asarray(bv, np.float32).reshape(G, D),
        np.asarray(Wo, np.float32).reshape(G, D, D),
    )
    cvec = wsum @ pb + np.asarray(bo, np.float32)
    out += cvec[:, None, :]
    return out


# revision 16
# speedup vs baseline: 2.2140x; 1.1147x over previous
"""Trainium2 Bass kernel for nn_GroupedQueryAttention_86380382257377.

Math: the reference einsums collapse.
  scores[b,q,h,g] = x[b,q,:] . (Wq_blk[g,h] @ ksum[b,g]) / sqrt(D)
  with ksum[b,g,:] = sum_s k[b,s,g,:];  weights = softmax_g(scores)
  out[b] = x[b] @ M[b] + cvec[b],
      M[b]   = sum_g wsum[b,g] * (Wv_g @ Wo_g),
      wsum[b,g] = sum_{q,h} weights[b,q,h,g]
      cvec[b]= sum_g wsum[b,g] * (bv_g @ Wo_g) + bo  (host-applied, exact).

Sharding: core c owns group c for Wq/Wk (one small AllGather of the
wq_eff vectors) and output columns [c*64,(c+1)*64) for Wv@Wo / x@M
(x, Wv replicated).

Precision: the scores path runs fp8(e4m3) with host-side scaling
(Wq*256 dodges subnormals, ksum/8 and wqe/64 dodge overflow; the exp
scale absorbs the net factor). Stage J / P use bf16. fp32 PSUM
everywhere. Verified ~7e-3 rel err in numpy (gate 2e-2).
"""

import numpy as np
import ml_dtypes

B, S, D, G, H = 2, 2048, 512, 8, 4
N_CORES = 8
FSL = D // N_CORES  # 64
P = 128
DC = D // P  # 4
JC = S // P  # 16
ACBH = DC * B * H  # 32
WQE = P * ACBH  # 4096
CHUNK = WQE + B * H + 8  # 4112 elems, 32B-aligned in bf16
INV_SQRT_D = 1.0 / float(np.sqrt(D))
EXP_SCALE = 2.0 * INV_SQRT_D  # wqe8 carries wq_eff/2
WQ_UP = 256.0
KS_DN = 1.0 / 8.0
WQE_DN = 1.0 / 64.0

_cache = {}


def _build_nc():
    import concourse.bass as bass
    import concourse.mybir as mybir
    import concourse.tile as tile
    from concourse import bacc

    fp = mybir.dt.float32
    bf = mybir.dt.bfloat16
    f8 = mybir.dt.float8e4
    nc = bacc.Bacc(None, num_devices=N_CORES)

    # ---- kernel I/O (per-core views, host-prepared) ----
    x8_d = nc.dram_tensor("x8T", [D, B, S], f8, kind="ExternalInput")      # [a,b,s]
    xb_d = nc.dram_tensor("xbT", [D, B, S], bf, kind="ExternalInput")      # [a,b,s]
    wq_d = nc.dram_tensor("wq8T", [D, H, D], f8, kind="ExternalInput")     # [e,h,a]
    wk_d = nc.dram_tensor("wk", [D, D], bf, kind="ExternalInput")          # [d,e]
    wv_d = nc.dram_tensor("wvT", [G, D, D], bf, kind="ExternalInput")      # [g,e,a]
    wo_d = nc.dram_tensor("wo_img", [P, G * DC * FSL], bf, kind="ExternalInput")
    bk_d = nc.dram_tensor("bk_img", [P, DC], fp, kind="ExternalInput")     # *S
    bq_d = nc.dram_tensor("bq_img", [P, DC * H], fp, kind="ExternalInput")
    out_d = nc.dram_tensor("out2", [B, S, FSL], bf, kind="ExternalOutput")
    ws_d = nc.dram_tensor("wsum_out", [B, G], fp, kind="ExternalOutput")

    with tile.TileContext(nc) as tc:
        with (
            tc.tile_pool(name="sing", bufs=1) as sing,
            tc.tile_pool(name="pk", bufs=1, space="PSUM") as pk,
            tc.tile_pool(name="ps", bufs=2, space="PSUM") as ps,
            tc.tile_pool(name="pw", bufs=1, space="PSUM") as pw,
            tc.tile_pool(name="ppP", bufs=2, space="PSUM") as ppP,
            tc.tile_pool(name="po", bufs=2, space="PSUM") as po,
            tc.tile_pool(name="dram", bufs=1, space="DRAM") as dram,
        ):
            # ---- persistent SBUF tiles ----
            x8_sb = sing.tile([P, DC, B, S], f8)         # 16KB/part
            xb_sb = sing.tile([P, DC, B, S], bf)         # 32KB/part
            wq_sb = sing.tile([P, DC, H, D], f8)         # 8KB/part
            wk_sb = sing.tile([P, DC, D], bf)            # 4KB/part
            wv_sb = sing.tile([P, G, DC, D], bf)         # 32KB/part
            wo_sb = sing.tile([P, G, DC, FSL], bf)       # 4KB/part
            bk_sb = sing.tile([P, DC], fp)
            bq_sb = sing.tile([P, DC, H], fp)
            red8 = sing.tile([P, S], f8)                 # Act-accum scratch
            xs_f = sing.tile([P, DC, B], fp)
            xs_bf = sing.tile([P, DC, B], bf)
            ksum_f = sing.tile([P, DC, B], fp)
            ksum8 = sing.tile([P, DC, B], f8)
            wqe_loc = sing.tile([P, DC, B, H], bf)
            bqd_loc = sing.tile([B, H + 4], bf)  # cols 4-7 zero-pad CHUNK
            wqe_all = sing.tile([P, G, DC, B, H], bf)
            wqe8 = sing.tile([P, G, DC, B, H], f8)
            bqd1 = sing.tile([1, G, B, H], bf)
            ebqd1 = sing.tile([1, B, H, G], bf)
            ebqd_bc = sing.tile([P, B, H, G], bf)
            e_sb = sing.tile([P, B, JC, H, G], bf)
            e2_sb = sing.tile([P, B, JC, H, G], bf)
            den_sb = sing.tile([P, B, JC, H], bf)
            rec_sb = sing.tile([P, B, JC, H], bf)
            ws_sb = sing.tile([1, B, G], fp)
            wsum_bc = sing.tile([P, B, G], fp)
            p_sb = sing.tile([P, DC, G, FSL], bf)
            m_sb = sing.tile([P, DC, B, FSL], bf)
            out_sb = sing.tile([P, JC, B, FSL], bf)      # [s%128, j, b, f]
            ones_f = sing.tile([1, P], fp)
            ones_b = sing.tile([1, P], bf)

            bounce = dram.tile([CHUNK], bf)
            gath = dram.tile([G * CHUNK], bf)

            nc.vector.memset(ones_f[:, :], 1.0)
            nc.vector.memset(bqd_loc[:, :], 0.0)
            nc.vector.memset(ones_b[:, :], 1.0)

            # ---- input DMAs on SP queue (program order = priority) ----
            nc.sync.dma_start(
                out=wk_sb[:, :, :], in_=wk_d.rearrange("(dc p) e -> p dc e", p=P)
            )
            nc.sync.dma_start(out=bk_sb[:, :], in_=bk_d[:, :])
            nc.sync.dma_start(
                out=bq_sb[:, :, :],
                in_=bq_d.rearrange("p (ec h) -> p ec h", ec=DC),
            )
            for dc in range(DC):
                for b in range(B):
                    nc.sync.dma_start(
                        out=x8_sb[:, dc, b, :],
                        in_=x8_d[dc * P:(dc + 1) * P, b, :],
                    )
            for ec in range(DC):
                nc.sync.dma_start(
                    out=wq_sb[:, ec, :, :], in_=wq_d[ec * P:(ec + 1) * P, :, :]
                )
            nc.sync.dma_start(
                out=wo_sb[:, :, :, :],
                in_=wo_d.rearrange("p (g ec f) -> p g ec f", g=G, ec=DC),
            )
            for g in range(G):
                for eh in range(2):
                    nc.sync.dma_start(
                        out=wv_sb[:, g, 2 * eh:2 * eh + 2, :],
                        in_=wv_d[g, 256 * eh:256 * (eh + 1), :].rearrange(
                            "(ec p) a -> p ec a", p=P
                        ),
                    )
            for b in range(B):
                for dc in range(DC):
                    for sh in range(2):
                        nc.sync.dma_start(
                            out=xb_sb[:, dc, b, 1024 * sh:1024 * (sh + 1)],
                            in_=xb_d[
                                dc * P:(dc + 1) * P, b, 1024 * sh:1024 * (sh + 1)
                            ],
                        )

            # ---- A. xs[a,b] = sum_s x8 across DVE (fused ttr) / Act lanes ----
            ttr_scr = sing.tile([P, S // 2], bf)  # ttr dummy out
            def reduce_unit(eng, dc, b, slot):
                if eng == "act":
                    nc.scalar.activation(
                        out=red8[:, :], in_=x8_sb[:, dc, b, :],
                        func=mybir.ActivationFunctionType.Copy,
                        accum_out=slot,
                    )
                else:
                    nc.vector.tensor_reduce(
                        out=slot, in_=x8_sb[:, dc, b, :],
                        axis=mybir.AxisListType.X, op=mybir.AluOpType.add,
                    )

            for dc in range(DC):
                for b in range(B):
                    reduce_unit("act" if (dc + b) % 2 == 0 else "dve",
                                dc, b, xs_f[:, dc, b:b + 1])
            with nc.allow_low_precision(reason="bf16 xs for bf16 matmul"):
                nc.vector.tensor_copy(xs_bf[:, :, :], xs_f[:, :, :])

            # ---- B. ksumT[e,b] = Wk_c^T xs + S*bk ----
            psum_k = pk.tile([P, DC, B], fp, tag="pk")
            for ec in range(DC):
                for dc in range(DC):
                    nc.tensor.matmul(
                        psum_k[:, ec, :],
                        lhsT=wk_sb[:, dc, ec * P:(ec + 1) * P],
                        rhs=xs_bf[:, dc, :],
                        start=(dc == 0),
                        stop=(dc == DC - 1),
                    )
            for ec in range(DC):
                nc.vector.tensor_scalar_add(
                    ksum_f[:, ec, :], psum_k[:, ec, :], bk_sb[:, ec:ec + 1]
                )
            with nc.allow_low_precision(reason="fp8 ksum for fp8 matmul, /8"):
                nc.vector.tensor_scalar_mul(ksum8[:, :, :], ksum_f[:, :, :], KS_DN)

            # ---- C. wq_eff (x32 scaled) per (h, ac); bq_dot[b,h] exact ----
            psum_wq = pk.tile([P, H, DC, B], fp, tag="pk")
            for h in range(H):
                for ac in range(DC):
                    for ec in range(DC):
                        nc.tensor.matmul(
                            psum_wq[:, h, ac, :],
                            lhsT=wq_sb[:, ec, h, ac * P:(ac + 1) * P],
                            rhs=ksum8[:, ec, :],
                            start=(ec == 0),
                            stop=(ec == DC - 1),
                        )
            psum_bqd = pk.tile([B, H], fp, tag="pk")
            for ec in range(DC):
                nc.tensor.matmul(
                    psum_bqd[:, :],
                    lhsT=ksum_f[:, ec, :],
                    rhs=bq_sb[:, ec, :],
                    start=(ec == 0),
                    stop=(ec == DC - 1),
                )
            nc.vector.tensor_copy(
                wqe_loc[:, :, :, :].rearrange("p ac b h -> p h ac b"),
                psum_wq[:, :, :, :],
            )
            nc.vector.tensor_copy(bqd_loc[:, 0:H], psum_bqd[:, :])
            # bounce out + AllGather + spread (all on Pool queue, in order)
            nc.scalar.dma_start(
                out=bounce[0:WQE].rearrange("(p f) -> p f", p=P),
                in_=wqe_loc[:, :, :, :],
            )
            nc.scalar.dma_start(
                out=bounce[WQE:WQE + B * (H + 4)].rearrange(
                    "(b h) -> b h", b=B
                ),
                in_=bqd_loc[:, :],
            )
            nc.gpsimd.collective_compute(
                "AllGather",
                mybir.AluOpType.bypass,
                replica_groups=[list(range(N_CORES))],
                ins=[bounce[:].opt()],
                outs=[gath[:].opt()],
            )
            gap = gath[:]
            nc.scalar.dma_start(
                out=wqe_all[:, :, :, :, :].opt(),
                in_=bass.AP(
                    tensor=gap.tensor,
                    offset=gap.offset,
                    ap=[[ACBH, P], [CHUNK, G], [1, ACBH]],
                ),
            )
            nc.scalar.dma_start(
                out=bqd1[:, :, :, :].opt(),
                in_=bass.AP(
                    tensor=gap.tensor,
                    offset=gap.offset + WQE,
                    ap=[[0, 1], [CHUNK, G], [1, B * H]],
                ),
            )
            with nc.allow_low_precision(reason="fp8 wqe for fp8 matmul, /64"):
                nc.vector.tensor_scalar_mul(
                    wqe8[:, :, :, :, :], wqe_all[:, :, :, :, :], WQE_DN
                )
            # ebqd = exp(bqd/sqrt(D)); broadcast to partitions via PE
            nc.scalar.activation(
                out=ebqd1[0:1, :, :, :],
                in_=bqd1[0:1, :, :, :].rearrange("p g b h -> p b h g"),
                func=mybir.ActivationFunctionType.Exp,
                scale=INV_SQRT_D,
            )
            psum_eb = pk.tile([P, B * H * G], fp, tag="pk")
            nc.tensor.matmul(
                psum_eb[:, :],
                lhsT=ones_b[0:1, :],
                rhs=ebqd1[0:1, :, :, :],
                start=True,
                stop=True,
            )
            nc.vector.tensor_copy(
                ebqd_bc[:, :, :, :],
                psum_eb[:, :].rearrange("p (b h g) -> p b h g", b=B, h=H),
            )

            # ---- G. P_g = Wv_g @ Wo_g[:, fsl] (fills the AllGather window) ----
            for g in range(G):
                psum_pg = ppP.tile([P, DC * FSL], fp, name="psum_pg")
                for ac in range(DC):
                    for ec in range(DC):
                        nc.tensor.matmul(
                            psum_pg[:, ac * FSL:(ac + 1) * FSL],
                            lhsT=wv_sb[:, g, ec, ac * P:(ac + 1) * P],
                            rhs=wo_sb[:, g, ec, :],
                            start=(ec == 0),
                            stop=(ec == DC - 1),
                        )
                pv = psum_pg[:, :].rearrange("p (ac f) -> p ac f", ac=DC)
                if g % 2 == 0:
                    nc.scalar.activation(
                        out=p_sb[:, :, g, :],
                        in_=pv,
                        func=mybir.ActivationFunctionType.Copy,
                    )
                else:
                    nc.vector.tensor_copy(p_sb[:, :, g, :], pv)

            # ---- F. scores + softmax + wsum, then M and out, per b ----
            psum_s = [
                ps.tile([P, JC, H, G], fp, tag="s", name=f"psum_s{b}")
                for b in range(B)
            ]
            psum_ws = [
                pw.tile([1, G], fp, tag="w", name=f"psum_ws{b}")
                for b in range(B)
            ]
            psum_wsb = [
                pk.tile([P, G], fp, tag="pk", name=f"psum_wsb{b}")
                for b in range(B)
            ]

            for b in range(B):
                for j in range(JC):
                    for dc in range(DC):
                        nc.tensor.matmul(
                            psum_s[b][:, j, :, :],
                            lhsT=x8_sb[:, dc, b, j * P:(j + 1) * P],
                            rhs=wqe8[:, :, dc, b, :].rearrange("p g h -> p h g"),
                            start=(dc == 0),
                            stop=(dc == DC - 1),
                        )
            for b in range(B):
                nc.scalar.activation(
                    out=e_sb[:, b, :, :, :],
                    in_=psum_s[b][:, :, :, :],
                    func=mybir.ActivationFunctionType.Exp,
                    scale=EXP_SCALE,
                )
                eb = ebqd_bc[:, b, :, :]
                nc.vector.tensor_tensor(
                    out=e2_sb[:, b, :, :, :],
                    in0=e_sb[:, b, :, :, :],
                    in1=bass.AP(
                        tensor=eb.tensor,
                        offset=eb.offset,
                        ap=[list(eb.ap[0]), [0, JC]] + list(eb.ap[1:]),
                    ),
                    op=mybir.AluOpType.mult,
                )
                with nc.allow_low_precision(reason="bf16 softmax den, ~7e-3 ok"):
                    nc.vector.tensor_reduce(
                        out=den_sb[:, b, :, :],
                        in_=e2_sb[:, b, :, :, :],
                        axis=mybir.AxisListType.X,
                        op=mybir.AluOpType.add,
                    )
                    nc.vector.reciprocal(rec_sb[:, b, :, :], den_sb[:, b, :, :])

            for b in range(B):
                # wsum[g] = sum_{p,j,h} rec[p,j,h] * e2[p,j,h,g] via PE accum
                n = 0
                for j in range(JC):
                    for h in range(H):
                        nc.tensor.matmul(
                            psum_ws[b][:, :],
                            lhsT=rec_sb[:, b, j, h:h + 1],
                            rhs=e2_sb[:, b, j, h, :],
                            start=(n == 0),
                            stop=(n == JC * H - 1),
                        )
                        n += 1
                nc.vector.tensor_copy(ws_sb[:, b, :], psum_ws[b][:, :])
                nc.tensor.matmul(
                    psum_wsb[b][:, :],
                    lhsT=ones_f[0:1, :],
                    rhs=ws_sb[0:1, b, :],
                    start=True,
                    stop=True,
                )
                nc.vector.tensor_copy(wsum_bc[:, b, :], psum_wsb[b][:, :])

                # ---- H. M[b] = sum_g wsum[b,g] * P_g ----
                nc.vector.tensor_scalar_mul(
                    m_sb[:, :, b, :], p_sb[:, :, 0, :], wsum_bc[:, b, 0:1]
                )
                for g in range(1, G):
                    nc.vector.scalar_tensor_tensor(
                        out=m_sb[:, :, b, :],
                        in0=p_sb[:, :, g, :],
                        scalar=wsum_bc[:, b, g:g + 1],
                        in1=m_sb[:, :, b, :],
                        op0=mybir.AluOpType.mult,
                        op1=mybir.AluOpType.add,
                    )

                # ---- J. out[b, s, f] = x[b] @ M[b], s-chunked [128, 64] ----
                for j in range(JC):
                    psum_o = po.tile([P, FSL], fp, name="psum_o")
                    for dc in range(DC):
                        nc.tensor.matmul(
                            psum_o[:, :],
                            lhsT=xb_sb[:, dc, b, j * P:(j + 1) * P],
                            rhs=m_sb[:, dc, b, :],
                            start=(dc == 0),
                            stop=(dc == DC - 1),
                        )
                    if j % 2 == 0:
                        nc.scalar.activation(
                            out=out_sb[:, j, b, :],
                            in_=psum_o[:, :],
                            func=mybir.ActivationFunctionType.Copy,
                        )
                    else:
                        nc.vector.tensor_copy(out_sb[:, j, b, :], psum_o[:, :])
                nc.sync.dma_start(
                    out=out_d[b, :, :].rearrange("(j p) f -> p j f", p=P),
                    in_=out_sb[:, :, b, :],
                )
            nc.sync.dma_start(out=ws_d[:, :], in_=ws_sb[0, :, :])

    nc.compile()
    return nc


def kernel(x, Wq, bq, Wk, bk, Wv, bv, Wo, bo):
    from concourse.bass_utils import run_bass_kernel_spmd

    if "nc" not in _cache:
        _cache["nc"] = _build_nc()
    nc = _cache["nc"]

    bft = ml_dtypes.bfloat16
    f8t = ml_dtypes.float8_e4m3fn
    x = np.ascontiguousarray(x, dtype=np.float32)
    xT = np.ascontiguousarray(x.transpose(2, 0, 1))                    # [D,B,S]
    x8T = xT.astype(f8t)
    xbT = xT.astype(bft)
    wvT = np.ascontiguousarray(
        Wv.astype(np.float32).reshape(D, G, D).transpose(1, 2, 0)      # [g,e,a]
    ).astype(bft)
    wo_r = Wo.astype(np.float32).reshape(G, D, D)
    wq_r = Wq.astype(np.float32).reshape(D, G, H, D)
    bq_r = np.asarray(bq, dtype=np.float32).reshape(G, H, D)
    bk_r = np.asarray(bk, dtype=np.float32).reshape(G, D)
    in_maps = []
    for c in range(N_CORES):
        fs = slice(c * FSL, (c + 1) * FSL)
        wo_img = np.ascontiguousarray(
            wo_r[:, :, fs].reshape(G, DC, P, FSL).transpose(2, 0, 1, 3)
        ).reshape(P, G * DC * FSL).astype(bft)
        bq_img = np.ascontiguousarray(
            bq_r[c].transpose(1, 0).reshape(DC, P, H).transpose(1, 0, 2)
        ).reshape(P, DC * H)  # [p, ec, h], e = ec*128+p
        bk_img = np.ascontiguousarray(bk_r[c].reshape(DC, P).T) * float(S)
        in_maps.append({
            "x8T": x8T,
            "xbT": xbT,
            "wq8T": np.ascontiguousarray(
                (WQ_UP * wq_r[:, c]).transpose(2, 1, 0)
            ).astype(f8t),
            "wk": np.ascontiguousarray(
                Wk[:, c * D:(c + 1) * D].astype(np.float32)
            ).astype(bft),
            "wvT": wvT,
            "wo_img": wo_img,
            "bk_img": np.ascontiguousarray(bk_img),
            "bq_img": bq_img,
        })
    res = run_bass_kernel_spmd(nc, in_maps, core_ids=list(range(N_CORES)))
    _cache["last_results"] = res
    outs = [r["out2"].astype(np.float32) for r in res.results]  # [B,S,FSL] each
    out = np.ascontiguousarray(np.concatenate(outs, axis=2))    # [B, S, D]
    # bias correction (exact; zero when bv == bo == 0)
    wsum = res.results[0]["wsum_out"].astype(np.float32)        # [B, G]
    pb = np.einsum(
        "gd,gdf->gf",
        np.asarray(bv, np.float32).reshape(G, D),
        np.asarray(Wo, np.float32).reshape(G, D, D),
    )
    cvec = wsum @ pb + np.asarray(bo, np.float32)
    out += cvec[:, None, :]
    return out
